# revision 1
# baseline (speedup 1.0000x reference)
"""Trainium2 Bass kernel for nn_DSWNV_84387517432212 (WaveNet-vocoder dense CNN).

Sharding: 8 cores = 4 batches x 2 time-halves. Each core computes output
t in [t0, t0+4125) for its (batch, half) from audio[t0-655, t0+4150) with a
650-sample receptive-field halo (zeros + masked conditioning reproduce the
reference's zero padding exactly on the first half).

Device algorithm (per core):
  A) conditioning: scale_in -> aux0 -> aux1 convs on 75 frames, then
     yaT[l] = (a2.T @ inx_w[l].T) with a cb row prepended; the x-upsample +
     inx 1x1 conv collapses into one K=76 matmul against a host-built
     "selector" matrix (row 0 = validity mask, rows 1..75 = aux frames
     scaled by up_w).
  B) causal conv (k=6) + softsign -> h0 over the whole extended range.
  C) 9 gated dilated-conv layers, chunked over time. h channels 128..191
     are stored twice in a 128-partition tile (upper half shifted left by
     the next layer's dilation) so the k=6 dilated conv runs as 9 full
     K=128 contraction chunks instead of 12. Skip contributions accumulate
     into an SBUF fp32 buffer; per-chunk tails round-trip through DRAM.
  D) relu -> out1 1x1 -> relu -> out2 computed transposed (time on PSUM
     partitions) so the (T, 256) output DMAs out contiguously.

All matmul operands are float32r (fp32 rounded to 11-bit mantissa, full PE
rate); producers write f32r natively (DMA / ACT / DVE-out). DVE cannot
*decode* f32r inputs, but f32r bits are plain rounded fp32, so DVE readers
use .bitcast(F32) (readers are not checked by the walrus f32r verifier;
writers are).
"""

import numpy as np

import concourse.mybir as mybir
import concourse.tile as tile
from concourse import bacc
from concourse.bass_utils import run_bass_kernel_spmd

F32 = mybir.dt.float32
F32R = mybir.dt.float32r
AF = mybir.ActivationFunctionType
ALU = mybir.AluOpType

# model dims
B, T, NQ, HID, KK, UP, TAUX, NCOND = 4, 8249, 256, 192, 6, 110, 75, 486
DILS = [1, 6, 36, 1, 6, 36, 1, 6, 36]
# sharding / tiling
TSH = 4125           # output cols per core
HALO = 650           # receptive field
EXT = 4800           # extended range (HALO + TSH + 25 pad)
APAD = 5             # causal conv left taps
TAIL = 180           # max layer lookback 5*36
CHUNKS = [(i * 960, 960) for i in range(5)]
NS = 320             # matmul subtile (>=256 keeps f32r at full rate)
OUT0, OUT1 = HALO, HALO + TSH   # valid h-cols for output

# xh channel permutation: [z 0:128 | t 0:128 | z 128:192 | t 128:192]
PERM = list(range(0, 128)) + list(range(192, 320)) + \
    list(range(128, 192)) + list(range(320, 384))


def _pack_weights(inp):
    """Host-side weight packing into SBUF-layout arrays (fp32)."""
    w = {}
    causal_w = inp["causal_w"]          # (192, 256, 6)
    w["wc"] = np.ascontiguousarray(np.stack(
        [causal_w[:, rc * 128:(rc + 1) * 128, tap].T
         for tap in range(6) for rc in range(2)], axis=1))          # (128,12,192)
    w["bc_row"] = inp["causal_b"].reshape(1, HID).copy()            # (1,192)

    dil = inp["dilh_w"][:, PERM, :, :]  # (9, 384, 192, 6)
    w["wd01"] = np.ascontiguousarray(
        dil[:, :, 0:128, :].transpose(0, 2, 3, 1))                  # (9,128,6,384)
    d2 = dil[:, :, 128:192, :].transpose(0, 2, 3, 1)                # (9,64,6,384)
    w["wd2p"] = np.ascontiguousarray(np.concatenate(
        [d2[:, :, 0::2, :], d2[:, :, 1::2, :]], axis=1))            # (9,128,3,384)
    bd = inp["dilh_b"][:, PERM]         # (9, 384)
    w["bd"] = np.ascontiguousarray(
        bd.reshape(9, 3, 128).transpose(2, 0, 1))                   # (128,9,3)

    sk = inp["skip_w"][:, :, :, 0]      # (9, 256, 192)
    w["ws01"] = np.ascontiguousarray(sk[:, :, 0:128].transpose(2, 0, 1))  # (128,9,256)
    w["ws2"] = np.ascontiguousarray(sk[:, :, 128:192].transpose(2, 0, 1))  # (64,9,256)
    w["bss"] = np.ascontiguousarray(
        inp["skip_b"].sum(0).reshape(2, 128).T)                     # (128,2)

    w["wsc"] = np.ascontiguousarray(inp["scale_in_w"][:, :, 0].T)   # (54,54)
    w["bsc"] = inp["scale_in_b"].reshape(54, 1).copy()
    w["wa0"] = np.ascontiguousarray(inp["aux0_w"].transpose(1, 2, 0))  # (54,3,162)
    b0 = np.zeros((128, 2), np.float32)
    b0.T.flat[:162] = inp["aux0_b"]
    w["ba0"] = b0
    a1t = inp["aux1_w"].transpose(1, 2, 0)                          # (162,3,486)
    w["wa1a"] = np.ascontiguousarray(a1t[0:128])                    # (128,3,486)
    w["wa1b"] = np.ascontiguousarray(a1t[128:162])                  # (34,3,486)
    b1 = np.zeros((128, 4), np.float32)
    b1.T.flat[:486] = inp["aux1_b"]
    w["ba1"] = b1

    inx = inp["inx_w"][:, :, :, 0][:, PERM, :]   # (9, 384, 486)
    wi = np.zeros((9, 4, 128, 384), np.float32)
    for r in range(4):
        n = min(128, 486 - r * 128)
        wi[:, r, :n, :] = inx[:, :, r * 128:r * 128 + n].transpose(0, 2, 1)
    w["wi"] = wi
    w["cb"] = np.ascontiguousarray(
        (inp["up_b"] * inx.sum(2) + inp["inx_b"][:, PERM])
        .reshape(9, 1, 384))                                        # (9,1,384)

    o1 = inp["out1_w"][:, :, 0]         # (256, 256)
    w["wo1"] = np.ascontiguousarray(
        o1.T.reshape(2, 128, 256).transpose(1, 0, 2))               # (128,2,256)
    w["bo1"] = np.ascontiguousarray(inp["out1_b"].reshape(2, 128).T)  # (128,2)
    o2 = inp["out2_w"][:, :, 0]
    w["wo2"] = np.ascontiguousarray(
        o2.T.reshape(2, 128, 256).transpose(1, 0, 2))               # (128,2,256)
    w["bo2row"] = inp["out2_b"].reshape(1, 256).copy()
    return w


def _per_core_arrays(inp, w, b, half):
    """Per-core input map (audio shard, selector, aux) + shared weights."""
    t0 = 0 if half == 0 else TSH
    audio = np.zeros((NQ, APAD + EXT), np.float32)
    g0 = t0 - HALO - APAD
    s0, s1 = max(0, g0), min(T, g0 + APAD + EXT)
    audio[:, s0 - g0:s1 - g0] = inp["audio"][b, :, s0:s1]

    # selector: rows 0..74 = aux frames scaled by up_w, row 75 = validity mask
    sel = np.zeros((76, EXT), np.float32)
    t = (t0 - HALO) + np.arange(EXT)
    valid = (t >= 0) & (t < T)
    tv = t[valid]
    sel[(tv + 1) // UP, np.where(valid)[0]] = inp["up_w"][(tv + 1) % UP]
    sel[75, valid] = 1.0

    m = {
        "audio_in": audio,
        "sel_in": sel,
        "mask_in": np.ascontiguousarray(sel[75:76, :]),
        "zeros180_in": np.zeros((128, TAIL), np.float32),
        "aux_in": np.ascontiguousarray(inp["aux"][b]),
    }
    for k, v in w.items():
        m[k + "_in"] = v
    return m


def build_kernel(reps=1, ablate=()):
    nc = bacc.Bacc(None, target_bir_lowering=False)
    d = {}
    shapes = {
        "audio_in": (NQ, APAD + EXT), "sel_in": (76, EXT),
        "mask_in": (1, EXT),
        "zeros180_in": (128, TAIL), "aux_in": (54, TAUX),
        "wc_in": (128, 12, HID), "bc_row_in": (1, HID),
        "wd01_in": (9, 128, 6, 384), "wd2p_in": (9, 128, 3, 384),
        "bd_in": (128, 9, 3),
        "ws01_in": (128, 9, 256), "ws2_in": (64, 9, 256), "bss_in": (128, 2),
        "wsc_in": (54, 54), "bsc_in": (54, 1),
        "wa0_in": (54, 3, 162), "ba0_in": (128, 2),
        "wa1a_in": (128, 3, NCOND), "wa1b_in": (34, 3, NCOND), "ba1_in": (128, 4),
        "wi_in": (9, 4, 128, 384), "cb_in": (9, 1, 384),
        "wo1_in": (128, 2, 256), "bo1_in": (128, 2),
        "wo2_in": (128, 2, 256), "bo2row_in": (1, 256),
    }
    for k, shp in shapes.items():
        d[k] = nc.dram_tensor(k, list(shp), F32, kind="ExternalInput")
    y_d = nc.dram_tensor("y", [TSH, NQ], F32, kind="ExternalOutput")
    tl01_d = nc.dram_tensor("tl01", [9, 128, TAIL], F32R, kind="Internal")
    tl2_d = nc.dram_tensor("tl2", [9, 128, TAIL], F32R, kind="Internal")

    def mm(out, lhsT, rhs, start, stop):
        if "pe" not in ablate:
            nc.tensor.matmul(out, lhsT, rhs, start=start, stop=stop)

    class _Skip:
        def __getattr__(self, name):
            return lambda *a, **k: None

    _skip = _Skip()

    def vec():
        return _skip if "dve" in ablate else nc.vector

    def act():
        return _skip if "act" in ablate else nc.scalar

    import contextlib

    with tile.TileContext(nc) as tc:
        rep_loop = tc.For_i(0, reps, 1) if reps > 1 else \
            contextlib.nullcontext()
        with rep_loop, tc.tile_pool(name="res", bufs=1) as res:
            sel_sb = res.tile([76, EXT], F32R)
            nc.sync.dma_start(out=sel_sb, in_=d["sel_in"][:, :].bitcast(F32R))
            mask_sb = res.tile([1, EXT], F32R)
            nc.sync.dma_start(out=mask_sb, in_=d["mask_in"][:, :].bitcast(F32R))
            h0_01 = res.tile([128, TAIL + EXT], F32R)
            h0_2 = res.tile([128, TAIL + EXT], F32R)
            for t_ in (h0_01, h0_2):
                nc.sync.dma_start(out=t_[:, 0:TAIL],
                                  in_=d["zeros180_in"][:, :].bitcast(F32R))
            yaT = [res.tile([76, 384], F32R, name=f"yaT{ll}") for ll in range(9)]
            ws01 = res.tile([128, 9, 256], F32R)
            nc.sync.dma_start(out=ws01, in_=d["ws01_in"][:, :, :].bitcast(F32R))
            ws2 = res.tile([64, 9, 256], F32R)
            nc.sync.dma_start(out=ws2, in_=d["ws2_in"][:, :, :].bitcast(F32R))
            bd_sb = res.tile([128, 9, 3], F32)
            nc.sync.dma_start(out=bd_sb, in_=d["bd_in"][:, :, :])
            bss_sb = res.tile([128, 2], F32)
            nc.sync.dma_start(out=bss_sb, in_=d["bss_in"][:, :])
            bo1_sb = res.tile([128, 2], F32)
            nc.sync.dma_start(out=bo1_sb, in_=d["bo1_in"][:, :])
            wo1 = res.tile([128, 2, 256], F32R)
            nc.sync.dma_start(out=wo1, in_=d["wo1_in"][:, :, :].bitcast(F32R))
            wo2 = res.tile([128, 2, 256], F32R)
            nc.sync.dma_start(out=wo2, in_=d["wo2_in"][:, :, :].bitcast(F32R))
            bo2row = res.tile([1, 256], F32R)
            nc.sync.dma_start(out=bo2row, in_=d["bo2row_in"][:, :].bitcast(F32R))

            # ---------------- Phase A: conditioning ----------------
            with tc.tile_pool(name="ca", bufs=1) as ca, \
                 tc.tile_pool(name="cw", bufs=2) as cw, \
                 tc.tile_pool(name="cp", bufs=1, space="PSUM") as cp:
                aux_sb = ca.tile([54, TAUX], F32)
                nc.sync.dma_start(out=aux_sb, in_=d["aux_in"][:, :])
                wsc = ca.tile([54, 54], F32)
                nc.sync.dma_start(out=wsc, in_=d["wsc_in"][:, :])
                bsc = ca.tile([54, 1], F32)
                nc.sync.dma_start(out=bsc, in_=d["bsc_in"][:, :])
                ba0 = ca.tile([128, 2], F32)
                nc.sync.dma_start(out=ba0, in_=d["ba0_in"][:, :])
                ba1 = ca.tile([128, 4], F32)
                nc.sync.dma_start(out=ba1, in_=d["ba1_in"][:, :])
                wa0 = ca.tile([54, 3, 162], F32)
                nc.sync.dma_start(out=wa0, in_=d["wa0_in"][:, :, :])
                wa1a = ca.tile([128, 3, NCOND], F32)
                nc.sync.dma_start(out=wa1a, in_=d["wa1a_in"][:, :, :])
                wa1b = ca.tile([34, 3, NCOND], F32)
                nc.sync.dma_start(out=wa1b, in_=d["wa1b_in"][:, :, :])

                a0p = cp.tile([54, TAUX], F32)
                mm(a0p, wsc, aux_sb, True, True)
                a0 = ca.tile([54, TAUX], F32)
                nc.scalar.activation(out=a0, in_=a0p, func=AF.Identity, bias=bsc)

                # aux0: k=3, dil=1, same-pad via partial-range accumulation
                a1blk = [(0, 128), (128, 34)]
                a1 = [ca.tile([wd, TAUX], F32, name=f"a1_{i}")
                      for i, (o0, wd) in enumerate(a1blk)]
                for i, (o0, wd) in enumerate(a1blk):
                    a1p = cp.tile([wd, TAUX], F32, name=f"a1p{i}", tag="a1p",
                                  bufs=2, padded_shape=[128, TAUX])
                    ls = wa0[:, :, o0:o0 + wd]
                    mm(a1p, ls[:, 1, :], a0, True, False)
                    mm(a1p[:, 1:TAUX], ls[:, 0, :], a0[:, 0:TAUX - 1],
                       False, False)
                    mm(a1p[:, 0:TAUX - 1], ls[:, 2, :], a0[:, 1:TAUX],
                       False, True)
                    nc.scalar.activation(out=a1[i], in_=a1p, func=AF.Identity,
                                         bias=ba0[0:wd, i:i + 1])

                # aux1: k=3, dil=3, same-pad
                a2blk = [(0, 128), (128, 128), (256, 128), (384, 102)]
                a2 = [ca.tile([wd, TAUX], F32R, name=f"a2_{i}")
                      for i, (o0, wd) in enumerate(a2blk)]
                for i, (o0, wd) in enumerate(a2blk):
                    a2p = cp.tile([wd, TAUX], F32, name=f"a2p{i}", tag="a2p",
                                  bufs=2, padded_shape=[128, TAUX])
                    for kc, wsrc in enumerate([wa1a, wa1b]):
                        ls = wsrc[:, :, o0:o0 + wd]
                        rhs = a1[kc]
                        mm(a2p, ls[:, 1, :], rhs, kc == 0, False)
                        mm(a2p[:, 3:TAUX], ls[:, 0, :], rhs[:, 0:TAUX - 3],
                           False, False)
                        mm(a2p[:, 0:TAUX - 3], ls[:, 2, :], rhs[:, 3:TAUX],
                           False, kc == 1)
                    nc.scalar.activation(out=a2[i], in_=a2p, func=AF.Identity,
                                         bias=ba1[0:wd, i:i + 1])

                # yaT[l] rows 1..75 = a2.T @ inx_w[l].T ; row 0 = cb
                for ll in range(9):
                    wi_sb = cw.tile([128, 4, 384], F32R, tag="wi")
                    nc.sync.dma_start(
                        out=wi_sb,
                        in_=d["wi_in"][ll, :, :, :].rearrange(
                            "r p n -> p r n").bitcast(F32R))
                    yp = cp.tile([TAUX, 384], F32, tag="yp", bufs=2)
                    for r, (o0, wd) in enumerate(a2blk):
                        mm(yp, a2[r], wi_sb[0:wd, r, :], r == 0, r == 3)
                    nc.scalar.activation(out=yaT[ll][0:TAUX, :], in_=yp,
                                         func=AF.Copy)
                    nc.sync.dma_start(out=yaT[ll][TAUX:76, :],
                                      in_=d["cb_in"][ll, :, :].bitcast(F32R))

            # ---------------- Phase B: causal conv + softsign ----------------
            with tc.tile_pool(name="pb", bufs=1) as pb, \
                 tc.tile_pool(name="pbs", bufs=2) as pbs, \
                 tc.tile_pool(name="pbp", bufs=3, space="PSUM") as pbp:
                a_t = [pb.tile([128, APAD + EXT], F32R, name=f"aud{i}")
                       for i in range(2)]
                for i in range(2):
                    nc.sync.dma_start(
                        out=a_t[i],
                        in_=d["audio_in"][i * 128:(i + 1) * 128, :].bitcast(F32R))
                wc = pb.tile([128, 12, HID], F32R)
                nc.sync.dma_start(out=wc, in_=d["wc_in"][:, :, :].bitcast(F32R))
                bcrow = pb.tile([1, HID], F32R)
                nc.sync.dma_start(out=bcrow, in_=d["bc_row_in"][:, :].bitcast(F32R))

                NB = 400
                cblk = [(0, 128), (128, 64)]
                for st in range(EXT // NB):
                    ccp = [pbp.tile([wd, NB], F32, tag=f"cc{i}",
                                    name=f"ccp{st}_{i}")
                           for i, (o0, wd) in enumerate(cblk)]
                    for i, (o0, wd) in enumerate(cblk):
                        for tap in range(6):
                            for rc in range(2):
                                mm(ccp[i],
                                   wc[:, tap * 2 + rc, o0:o0 + wd],
                                   a_t[rc][:, st * NB + tap:st * NB + tap + NB],
                                   tap == 0 and rc == 0, False)
                        mm(ccp[i], bcrow[:, o0:o0 + wd],
                           mask_sb[:, st * NB:(st + 1) * NB], False, True)
                    for i, (o0, wd) in enumerate(cblk):
                        den = pbs.tile([wd, NB], F32, tag=f"ab{i}")
                        nc.scalar.activation(out=den, in_=ccp[i], func=AF.Abs)
                        nc.vector.tensor_scalar(
                            out=den, in0=den, scalar1=1.0, scalar2=None,
                            op0=ALU.add)
                        rr = pbs.tile([wd, NB], F32, tag=f"rr{i}")
                        nc.vector.reciprocal_approx_fast(out=rr, in_=den)
                        dst = h0_01 if i == 0 else h0_2
                        nc.vector.tensor_tensor(
                            out=dst[0:wd, TAIL + st * NB:TAIL + (st + 1) * NB],
                            in0=ccp[i], in1=rr, op=ALU.mult)
                    # shifted upper copy of h0 ch 128..191 (layer 0 dil = 1)
                    nc.sync.dma_start(
                        out=h0_2[64:128,
                                 TAIL + st * NB - 1:TAIL + (st + 1) * NB - 1],
                        in_=h0_2[0:64,
                                 TAIL + st * NB:TAIL + (st + 1) * NB])

            # ---------------- Phase C: 9 gated layers, chunked ----------------
            with tc.tile_pool(name="hw", bufs=2) as hw, \
                 tc.tile_pool(name="wdp", bufs=2) as wdp, \
                 tc.tile_pool(name="scr", bufs=2) as scr, \
                 tc.tile_pool(name="ssp", bufs=1) as ssp, \
                 tc.tile_pool(name="od", bufs=1) as od, \
                 tc.tile_pool(name="pc", bufs=1, space="PSUM") as pc:
                for c, (c0, chw) in enumerate(CHUNKS):
                    ss = [ssp.tile([128, chw], F32, tag=f"ss{i}",
                                   name=f"ss{c}_{i}") for i in range(2)]
                    prev01, prev2, poff = h0_01, h0_2, TAIL + c0

                    for ll in range(9):
                        dil = DILS[ll]
                        dnx = DILS[ll + 1] if ll < 8 else 1
                        cur01 = hw.tile([128, TAIL + chw], F32R, tag="h01")
                        cur2 = hw.tile([128, TAIL + chw], F32R, tag="h2")
                        if c == 0:
                            for t_ in (cur01, cur2):
                                nc.sync.dma_start(
                                    out=t_[:, 0:TAIL],
                                    in_=d["zeros180_in"][:, :].bitcast(F32R))
                        else:
                            nc.sync.dma_start(out=cur01[:, 0:TAIL],
                                              in_=tl01_d[ll, :, :])
                            nc.sync.dma_start(out=cur2[:, 0:TAIL],
                                              in_=tl2_d[ll, :, :])
                        wd01 = wdp.tile([128, 6, 384], F32R, tag="wd01")
                        wd2p = wdp.tile([128, 3, 384], F32R, tag="wd2p")
                        if "wdma" not in ablate:
                            nc.sync.dma_start(out=wd01,
                                              in_=d["wd01_in"][ll, :, :, :]
                                              .bitcast(F32R))
                            nc.sync.dma_start(out=wd2p,
                                              in_=d["wd2p_in"][ll, :, :, :]
                                              .bitcast(F32R))

                        for st in range(chw // NS):
                            sb_ = c0 + st * NS       # h-range col of subtile
                            lb = poff + st * NS      # col in prev buffers
                            xcs = []
                            for mb in range(3):
                                xcp = pc.tile([128, NS], F32, tag="xc",
                                              bufs=2, name=f"xcp{mb}")
                                mm(xcp, yaT[ll][:, mb * 128:(mb + 1) * 128],
                                   sel_sb[:, sb_:sb_ + NS], True, True)
                                xc_sb = scr.tile([128, NS], F32, tag=f"xcs{mb}")
                                act().activation(out=xc_sb, in_=xcp,
                                                     func=AF.Copy)
                                xcs.append(xc_sb)
                            xh = []
                            for mb in range(3):
                                hcp = pc.tile([128, NS], F32, tag="hc", bufs=4,
                                              name=f"hcp{mb}")
                                for tap in range(6):
                                    off = (tap - 5) * dil
                                    mm(hcp,
                                       wd01[:, tap, mb * 128:(mb + 1) * 128],
                                       prev01[:, lb + off:lb + off + NS],
                                       tap == 0, False)
                                for j in range(3):
                                    off = (2 * j - 5) * dil
                                    mm(hcp,
                                       wd2p[:, j, mb * 128:(mb + 1) * 128],
                                       prev2[:, lb + off:lb + off + NS],
                                       False, j == 2)
                                xh_sb = scr.tile([128, NS], F32, tag=f"xh{mb}")
                                vec().scalar_tensor_tensor(
                                    out=xh_sb, in0=hcp,
                                    scalar=bd_sb[:, ll, mb:mb + 1],
                                    in1=xcs[mb], op0=ALU.add, op1=ALU.mult)
                                xh.append(xh_sb)
                            # xh blocks (permuted): [z0:128], [t0:128],
                            #                       [z128:192 | t128:192]
                            xh2b = scr.tile([64, NS], F32, tag="xh2b",
                                            padded_shape=[128, NS])
                            nc.sync.dma_start(out=xh2b, in_=xh[2][64:128, :])
                            act().activation(out=xh[0], in_=xh[0],
                                                 func=AF.Sigmoid)
                            act().activation(out=xh[2][0:64, :],
                                                 in_=xh[2][0:64, :],
                                                 func=AF.Sigmoid)
                            act().activation(out=xh[1], in_=xh[1],
                                                 func=AF.Tanh)
                            act().activation(out=xh2b, in_=xh2b,
                                                 func=AF.Tanh)
                            wcol = TAIL + st * NS
                            for i, (zz, ttt, hp, cdst, wd_) in enumerate([
                                    (xh[0], xh[1],
                                     prev01[:, lb:lb + NS].bitcast(F32),
                                     cur01, 128),
                                    (xh[2][0:64, :], xh2b,
                                     prev2[0:64, lb:lb + NS].bitcast(F32),
                                     cur2, 64)]):
                                dd = scr.tile([wd_, NS], F32, tag=f"dd{i}",
                                              padded_shape=[128, NS])
                                vec().tensor_tensor(
                                    out=dd, in0=hp, in1=ttt, op=ALU.subtract)
                                vec().tensor_tensor(
                                    out=dd, in0=zz, in1=dd, op=ALU.mult)
                                vec().tensor_tensor(
                                    out=cdst[0:wd_, wcol:wcol + NS],
                                    in0=ttt, in1=dd, op=ALU.add)
                            # shifted upper copy for next layer's pair chunks
                            nc.sync.dma_start(
                                out=cur2[64:128, wcol - dnx:wcol - dnx + NS],
                                in_=cur2[0:64, wcol:wcol + NS])
                            # skip conv
                            for ob in range(2):
                                skp = pc.tile([128, NS], F32, tag="sk",
                                              bufs=2, name=f"skp{ob}")
                                mm(skp, ws01[:, ll, ob * 128:(ob + 1) * 128],
                                   cur01[:, wcol:wcol + NS], True, False)
                                mm(skp, ws2[:, ll, ob * 128:(ob + 1) * 128],
                                   cur2[0:64, wcol:wcol + NS], False, True)
                                if ll == 0:
                                    act().activation(
                                        out=ss[ob][:, st * NS:(st + 1) * NS],
                                        in_=skp, func=AF.Identity,
                                        bias=bss_sb[:, ob:ob + 1])
                                else:
                                    vec().tensor_tensor(
                                        out=ss[ob][:, st * NS:(st + 1) * NS],
                                        in0=skp,
                                        in1=ss[ob][:, st * NS:(st + 1) * NS],
                                        op=ALU.add)
                        if c < len(CHUNKS) - 1:
                            nc.sync.dma_start(out=tl01_d[ll, :, :],
                                              in_=cur01[:, chw:chw + TAIL])
                            nc.sync.dma_start(out=tl2_d[ll, :, :],
                                              in_=cur2[:, chw:chw + TAIL])
                        prev01, prev2, poff = cur01, cur2, TAIL

                    # ---------------- Phase D: output convs ----------------
                    os_, oe = max(c0, OUT0), min(c0 + chw, OUT1)
                    if os_ >= oe:
                        continue
                    r1 = [od.tile([128, chw], F32R, tag=f"r1{i}",
                                  name=f"r1{c}_{i}") for i in range(2)]
                    for q0 in range(((os_ - c0) // NS) * NS, oe - c0, NS):
                        rlq = []
                        for kc in range(2):
                            rt = scr.tile([128, NS], F32R, tag=f"rlq{kc}")
                            act().activation(out=rt,
                                                 in_=ss[kc][:, q0:q0 + NS],
                                                 func=AF.Relu)
                            rlq.append(rt)
                        for ob in range(2):
                            o1p = pc.tile([128, NS], F32, tag="xc", bufs=2,
                                          name=f"o1p{ob}")
                            for kc in range(2):
                                mm(o1p,
                                   wo1[:, kc, ob * 128:(ob + 1) * 128],
                                   rlq[kc], kc == 0, kc == 1)
                            act().activation(
                                out=r1[ob][:, q0:q0 + NS], in_=o1p,
                                func=AF.Relu, bias=bo1_sb[:, ob:ob + 1])
                    for q0 in range(os_, oe, 128):
                        qw = min(128, oe - q0)
                        o2p = pc.tile([128, 256], F32, tag="sk", bufs=2,
                                      name="o2p")
                        mm(o2p[0:qw, :], r1[0][:, q0 - c0:q0 - c0 + qw],
                           wo2[:, 0, :], True, False)
                        mm(o2p[0:qw, :], r1[1][:, q0 - c0:q0 - c0 + qw],
                           wo2[:, 1, :], False, False)
                        mm(o2p[0:qw, :], mask_sb[:, q0:q0 + qw], bo2row,
                           False, True)
                        og = od.tile([128, 256], F32, tag="og", bufs=2)
                        act().activation(out=og[0:qw, :], in_=o2p[0:qw, :],
                                             func=AF.Copy)
                        nc.sync.dma_start(out=y_d[q0 - OUT0:q0 - OUT0 + qw, :],
                                          in_=og[0:qw, :])
    nc.compile()
    return nc


_NC_CACHE = {}


def kernel(**inputs):
    inp = {k: np.ascontiguousarray(np.asarray(v, dtype=np.float32))
           for k, v in inputs.items()}
    if "nc" not in _NC_CACHE:
        _NC_CACHE["nc"] = build_kernel()
    nc = _NC_CACHE["nc"]
    w = _pack_weights(inp)
    in_maps = [_per_core_arrays(inp, w, core // 2, core % 2)
               for core in range(8)]
    res = run_bass_kernel_spmd(nc, in_maps, core_ids=list(range(8)))
    out = np.empty((B, T, NQ), np.float32)
    for core in range(8):
        b, half = core // 2, core % 2
        y = res.results[core]["y"]
        if half == 0:
            out[b, 0:TSH] = y
        else:
            out[b, TSH:T] = y[0:T - TSH]
    return out

